# revision 1
# baseline (speedup 1.0000x reference)
"""MoE QKV parallel linear for Trainium2, 8 NeuronCores.

Problem: out[t] = x[t] @ W[id[t]].T with x [16384, 2048] f32,
W [4, 3072, 2048] f32, id sorted int32 (tokens pre-grouped by expert).

Sharding: data-parallel over tokens with expert-pure shards. Since tokens
are sorted by expert, split each expert's contiguous token range across a
proportional share of the 8 cores. Every core then runs one dense matmul
[T_max, 2048] @ [2048, 3072] against a single expert's weight (padded with
zero tokens up to the common T_max), which keeps the SPMD program uniform
across cores. Host transposes x-shards and W so the device kernel needs no
on-chip transposes, and scatters the per-core results back.

Device kernel (per core): x^T resident in SBUF (f32r), W^T streamed in
[128, 512] k-tiles, PE accumulates 16 k-tiles per [128 tok, 512 out] PSUM
tile using the fp32r fast path (1 col/cycle), DVE copies PSUM->SBUF,
HWDGE DMAs store to DRAM.
"""

import numpy as np

import concourse.bacc as bacc
import concourse.mybir as mybir
import concourse.tile as tile

NCORES = 8
HIDDEN = 2048
QKV_OUT = 3072
P = 128
KO = HIDDEN // P          # 16 contraction tiles
NCHUNK = 512              # PSUM free dim per matmul (fp32 max)
NCH = QKV_OUT // NCHUNK   # 6 output chunks
MB = 4                    # m-tiles per x DMA block (512 tokens)
SB_MT = 17                # max m-tiles of resident x^T (SBUF budget)

_cache: dict = {}


def _build(mt: int):
    """Bass module for one core: out[mt*128, 3072] = xT.T @ wT."""
    nc = bacc.Bacc("TRN2", target_bir_lowering=False, debug=False)
    tmax = mt * P
    f32r = mybir.dt.float32r
    f32 = mybir.dt.float32

    xT = nc.dram_tensor("xT", [HIDDEN, tmax], f32r, kind="ExternalInput")
    wT = nc.dram_tensor("wT", [HIDDEN, QKV_OUT], f32r, kind="ExternalInput")
    out = nc.dram_tensor("out", [tmax, QKV_OUT], f32, kind="ExternalOutput")

    # SBUF budget (192 KB/partition): resident x = sb_mt*8KB, W pool 2KB per
    # buf, out staging 8KB. Shrink W bufs if a skewed token split inflates mt.
    # mt > SB_MT (vanishingly rare skew) processes m in superblocks of 16
    # tiles, re-streaming W per superblock.
    sb_mt = mt if mt <= SB_MT else 16
    n_sb = -(-mt // sb_mt)
    tail = (mt - (n_sb - 1) * sb_mt) % MB  # only the last superblock is ragged
    w_bufs = max(17, min(23, (192 - 10 - sb_mt * 8) // 2))

    with tile.TileContext(nc) as tc:
        with (
            tc.tile_pool(name="xa", bufs=max(1, KO * (sb_mt // MB))) as xa,
            tc.tile_pool(name="xr", bufs=KO if tail else 1) as xr,
            tc.tile_pool(name="wp", bufs=w_bufs) as wp,
            tc.tile_pool(name="pp", bufs=8, space="PSUM") as pp,
            tc.tile_pool(name="op", bufs=4) as op,
        ):
            def load_w(n, sb, split=False):
                """Chunk n's W k-tiles. Normally on the scalar ring (so they
                never queue behind the x stream); the very first chunk is
                split across both HWDGE rings to land faster."""
                wts = []
                for ko in range(KO):
                    w = wp.tile([P, NCHUNK], f32r, name=f"w_{sb}_{n}_{ko}", tag="w")
                    eng = nc.sync if (split and ko >= KO // 2) else nc.scalar
                    eng.dma_start(
                        out=w[:],
                        in_=wT[ko * P:(ko + 1) * P, n * NCHUNK:(n + 1) * NCHUNK],
                    )
                    wts.append(w)
                return wts

            for sb in range(n_sb):
                m0 = sb * sb_mt
                smt = min(sb_mt, mt - m0)  # m-tiles in this superblock
                wq = {0: load_w(0, sb), 1: load_w(1, sb)}

                # resident x^T for this superblock: tiles[ko][mb] of
                # [128, MB*128] (+ ragged tail), emitted mb-major so the
                # sync ring delivers whole m-blocks in order.
                n_mb = -(-smt // MB)
                xt = [[None] * n_mb for _ in range(KO)]
                for mb in range(n_mb):
                    mw = min(MB, smt - mb * MB)
                    for ko in range(KO):
                        if mw == MB:
                            t = xa.tile([P, MB * P], f32r,
                                        name=f"x_{sb}_{ko}_{mb}", tag="x")
                        else:
                            t = xr.tile([P, mw * P], f32r,
                                        name=f"x_{sb}_{ko}_{mb}", tag="xr")
                        c0 = (m0 + mb * MB) * P
                        nc.sync.dma_start(
                            out=t[:],
                            in_=xT[ko * P:(ko + 1) * P, c0:c0 + mw * P],
                        )
                        xt[ko][mb] = t

                def x_slice(ko, mi):
                    mb, r = divmod(mi, MB)
                    return xt[ko][mb][:, r * P:(r + 1) * P]

                for n in range(NCH):
                    wts = wq.pop(n)
                    for mi in range(smt):
                        m = m0 + mi
                        ps = pp.tile([P, NCHUNK], f32, name=f"ps_{n}_{m}", tag="ps")
                        for ko in range(KO):
                            nc.tensor.matmul(
                                ps[:], x_slice(ko, mi), wts[ko][:],
                                start=(ko == 0), stop=(ko == KO - 1),
                            )
                        ot = op.tile([P, NCHUNK], f32, name=f"o_{n}_{m}", tag="o")
                        nc.vector.tensor_copy(ot[:], ps[:])
                        nc.scalar.dma_start(
                            out=out[m * P:(m + 1) * P,
                                    n * NCHUNK:(n + 1) * NCHUNK],
                            in_=ot[:],
                        )
                    if n + 2 < NCH:
                        wq[n + 2] = load_w(n + 2, sb)
    nc.compile()
    return nc


def _plan(counts):
    """Allocate 8 cores to experts proportionally (largest remainder),
    then split each expert's token range into per-core contiguous spans.
    Returns (spans, t_max): spans[c] = (expert, start, length)."""
    total = int(counts.sum())
    ne = len(counts)
    active = [e for e in range(ne) if counts[e] > 0]
    quota = {e: counts[e] * NCORES / total for e in active}
    alloc = {e: max(1, int(quota[e])) for e in active}
    while sum(alloc.values()) > NCORES:  # too many mins; shrink largest
        shrinkable = [e for e in active if alloc[e] > 1]
        e = max(shrinkable, key=lambda e: alloc[e] - quota[e])
        alloc[e] -= 1
    rema = sorted(active, key=lambda e: quota[e] - alloc[e], reverse=True)
    i = 0
    while sum(alloc.values()) < NCORES:
        alloc[rema[i % len(rema)]] += 1
        i += 1
    spans = []
    starts = np.concatenate([[0], np.cumsum(counts)])
    for e in active:
        k = alloc[e]
        base, extra = divmod(int(counts[e]), k)
        off = int(starts[e])
        for j in range(k):
            ln = base + (1 if j < extra else 0)
            spans.append((e, off, ln))
            off += ln
    t_max = max(ln for _, _, ln in spans)
    t_max = max(P, -(-t_max // P) * P)
    return spans, t_max


def _runner(mt: int):
    """Compiled 8-core executor for the mt-tile module, cached so repeat
    kernel() calls skip jax retracing. Mirrors bass2jax.run_bass_via_pjrt's
    multi-core path (concat per-core inputs on axis 0 + shard_map)."""
    import jax
    import jax.numpy as jnp
    from jax.sharding import Mesh, PartitionSpec
    from jax.experimental.shard_map import shard_map
    from concourse import bass2jax, mybir as mb

    nc = _build(mt)
    bass2jax.install_neuronx_cc_hook()

    part_name = nc.partition_id_tensor.name if nc.partition_id_tensor else None
    in_names, out_names, out_avals = [], [], []
    for alloc in nc.m.functions[0].allocations:
        if not isinstance(alloc, mb.MemoryLocationSet):
            continue
        name = alloc.memorylocations[0].name
        if alloc.kind == "ExternalInput":
            if name != part_name:
                in_names.append(name)
        elif alloc.kind == "ExternalOutput":
            out_names.append(name)
            out_avals.append(
                jax.core.ShapedArray(tuple(alloc.tensor_shape),
                                     mb.dt.np(alloc.dtype)))
    n_params = len(in_names)
    n_outs = len(out_names)
    bind_names = in_names + out_names + ([part_name] if part_name else [])

    def _body(*args):
        operands = list(args)
        if part_name:
            operands.append(bass2jax.partition_id_tensor())
        outs = bass2jax._bass_exec_p.bind(
            *operands,
            out_avals=tuple(out_avals),
            in_names=tuple(bind_names),
            out_names=tuple(out_names),
            lowering_input_output_aliases=(),
            sim_require_finite=True,
            sim_require_nnan=True,
            nc=nc,
        )
        return tuple(outs)

    devices = jax.devices()[:NCORES]
    mesh = Mesh(np.asarray(devices), ("core",))
    sharded = jax.jit(
        shard_map(_body, mesh=mesh,
                  in_specs=(PartitionSpec("core"),) * (n_params + n_outs),
                  out_specs=(PartitionSpec("core"),) * n_outs,
                  check_rep=False),
        donate_argnums=tuple(range(n_params, n_params + n_outs)),
        keep_unused=True,
    )

    def run(in_maps):
        concat_in = [
            np.concatenate([m[name] for m in in_maps], axis=0)
            for name in in_names
        ]
        zeros = [np.zeros((NCORES * a.shape[0], *a.shape[1:]), a.dtype)
                 for a in out_avals]
        outs = sharded(*concat_in, *zeros)
        return [
            {name: np.asarray(outs[i]).reshape(NCORES, *out_avals[i].shape)[c]
             for i, name in enumerate(out_names)}
            for c in range(NCORES)
        ]

    return run


def kernel(x, W, modality_mapping):
    x = np.ascontiguousarray(np.asarray(x, dtype=np.float32))
    W = np.asarray(W, dtype=np.float32)
    mm = np.asarray(modality_mapping)

    perm = None
    if np.any(np.diff(mm) < 0):  # insurance: tokens not pre-sorted
        perm = np.argsort(mm, kind="stable")
        x = x[perm]
        mm = mm[perm]

    T = x.shape[0]
    E = W.shape[0]
    counts = np.bincount(mm.astype(np.int64), minlength=E)
    spans, t_max = _plan(counts)
    mt = t_max // P

    if mt not in _cache:
        _cache[mt] = _runner(mt)
    run = _cache[mt]

    wTs = {}
    in_maps = []
    for e, off, ln in spans:
        if e not in wTs:
            wTs[e] = np.ascontiguousarray(W[e].T)
        xTp = np.zeros((HIDDEN, t_max), dtype=np.float32)
        xTp[:, :ln] = x[off:off + ln].T
        in_maps.append({"xT": xTp, "wT": wTs[e]})

    results = run(in_maps)

    out = np.empty((T, QKV_OUT), dtype=np.float32)
    for c, (e, off, ln) in enumerate(spans):
        out[off:off + ln] = results[c]["out"][:ln]
    if perm is not None:
        inv = np.empty_like(perm)
        inv[perm] = np.arange(T)
        out = out[inv]
    return out



# revision 2
# speedup vs baseline: 1.0859x; 1.0859x over previous
"""MoE QKV parallel linear for Trainium2, 8 NeuronCores.

Problem: out[t] = x[t] @ W[id[t]].T with x [16384, 2048] f32,
W [4, 3072, 2048] f32, id sorted int32 (tokens pre-grouped by expert).

Sharding: tensor-parallel over the QKV output dim (vLLM column-parallel
style). Each core owns a 384-column output shard and streams ALL tokens
through it, so the SPMD program is bit-identical across cores (only the
W slice differs) and expert imbalance costs nothing: ragged token
chunks stream at their actual length on the PE.

Device kernel (per core): W shard resident in SBUF as bf16 stationary
tiles [128k, 128m] reused across 4 consecutive 512-token streams (LDW
amortized 4x), x^T streamed in 2048-token blocks (16 k-tiles each,
double buffered), PE accumulates 16 k-tiles per [128 out, 512 tok]
PSUM bank, DVE copies PSUM->SBUF casting to bf16, out stored
transposed [384, 16384] so every DMA row is contiguous; the host
re-transposes. bf16 throughout: matmul streams 1 col/cycle, DMA and
LDWEIGHTS halve vs fp32, and abs-max rel err stays ~4e-3 (gate 2e-2).
"""

import numpy as np

import concourse.bacc as bacc
import concourse.mybir as mybir
import concourse.tile as tile

NCORES = 8
T = 16384
HIDDEN = 2048
QKV_OUT = 3072
NSH = QKV_OUT // NCORES   # 384 output cols per core
P = 128
KO = HIDDEN // P          # 16 contraction tiles
MT = NSH // P             # 3 stationary out-tiles per core
BLK = 2048                # tokens per x superblock (16 KB/part bf16)
CH = 512                  # tokens per matmul stream (PE max moving dim)

_cache: dict = {}


def _blocks(counts):
    """Compile-time schedule: contiguous expert-pure token blocks."""
    blocks = []
    off = 0
    for e, c in enumerate(counts):
        c = int(c)
        b0 = 0
        while b0 < c:
            blk = min(BLK, c - b0)
            blocks.append((e, off + b0, blk))
            b0 += blk
        off += c
    return blocks


def _build(counts):
    """One-core Bass module; identical program on all 8 cores."""
    nc = bacc.Bacc("TRN2", target_bir_lowering=False, debug=False)
    bf16 = mybir.dt.bfloat16
    f32 = mybir.dt.float32

    xT = nc.dram_tensor("xT", [HIDDEN, T], bf16, kind="ExternalInput")
    wT = nc.dram_tensor("wT", [4 * HIDDEN, NSH], bf16, kind="ExternalInput")
    out = nc.dram_tensor("out", [NSH, T], bf16, kind="ExternalOutput")

    blocks = _blocks(counts)
    experts = sorted({e for e, _, _ in blocks})

    with tile.TileContext(nc) as tc:
        with (
            tc.tile_pool(name="wp", bufs=KO * len(experts)) as wp,
            tc.tile_pool(name="xp", bufs=2 * KO) as xp,
            tc.tile_pool(name="pp", bufs=8, space="PSUM") as pp,
            tc.tile_pool(name="op", bufs=8) as op,
        ):
            # Resident W shard: [128, 384] per (expert, ko); ~48 KB/part.
            # First-needed expert first so compute starts ASAP.
            wt = {}
            first_e = blocks[0][0]
            for e in sorted(experts, key=lambda e: (e != first_e, e)):
                for ko in range(KO):
                    w = wp.tile([P, NSH], bf16, name=f"w_{e}_{ko}", tag="w")
                    nc.scalar.dma_start(
                        out=w[:],
                        in_=wT[(e * KO + ko) * P:(e * KO + ko + 1) * P, :],
                    )
                    wt[(e, ko)] = w

            for bi, (e, t0, blk) in enumerate(blocks):
                xb = []
                for ko in range(KO):
                    t = xp.tile([P, BLK], bf16, name=f"x_{bi}_{ko}", tag="x")
                    nc.sync.dma_start(
                        out=t[:, :blk],
                        in_=xT[ko * P:(ko + 1) * P, t0:t0 + blk],
                    )
                    xb.append(t)
                chunks = [(c0, min(CH, blk - c0)) for c0 in range(0, blk, CH)]

                for m in range(MT):
                    pss = [None] * len(chunks)
                    for ko in range(KO):
                        lhsT = wt[(e, ko)][:, m * P:(m + 1) * P]
                        for ci, (c0, cn) in enumerate(chunks):
                            if ko == 0:
                                pss[ci] = pp.tile([P, CH], f32,
                                                  name=f"ps_{bi}_{m}_{ci}",
                                                  tag="ps")
                            nc.tensor.matmul(
                                pss[ci][:, :cn], lhsT, xb[ko][:, c0:c0 + cn],
                                start=(ko == 0), stop=(ko == KO - 1),
                            )
                    for ci, (c0, cn) in enumerate(chunks):
                        ot = op.tile([P, CH], bf16, name=f"o_{bi}_{m}_{ci}",
                                     tag="o")
                        nc.vector.tensor_copy(ot[:, :cn], pss[ci][:, :cn])
                        nc.scalar.dma_start(
                            out=out[m * P:(m + 1) * P, t0 + c0:t0 + c0 + cn],
                            in_=ot[:, :cn],
                        )
    nc.compile()
    return nc


def _runner(counts):
    """Compiled 8-core executor, cached by expert counts. Mirrors
    bass2jax.run_bass_via_pjrt's multi-core path (concat per-core
    inputs on axis 0 + shard_map)."""
    import jax
    import jax.numpy as jnp
    from jax.sharding import Mesh, PartitionSpec
    from jax.experimental.shard_map import shard_map
    from concourse import bass2jax, mybir as mb

    nc = _build(counts)
    bass2jax.install_neuronx_cc_hook()

    part_name = nc.partition_id_tensor.name if nc.partition_id_tensor else None
    in_names, out_names, out_avals = [], [], []
    for alloc in nc.m.functions[0].allocations:
        if not isinstance(alloc, mb.MemoryLocationSet):
            continue
        name = alloc.memorylocations[0].name
        if alloc.kind == "ExternalInput":
            if name != part_name:
                in_names.append(name)
        elif alloc.kind == "ExternalOutput":
            out_names.append(name)
            out_avals.append(
                jax.core.ShapedArray(tuple(alloc.tensor_shape),
                                     mb.dt.np(alloc.dtype)))
    n_params = len(in_names)
    n_outs = len(out_names)
    bind_names = in_names + out_names + ([part_name] if part_name else [])

    def _body(*args):
        operands = list(args)
        if part_name:
            operands.append(bass2jax.partition_id_tensor())
        outs = bass2jax._bass_exec_p.bind(
            *operands,
            out_avals=tuple(out_avals),
            in_names=tuple(bind_names),
            out_names=tuple(out_names),
            lowering_input_output_aliases=(),
            sim_require_finite=True,
            sim_require_nnan=True,
            nc=nc,
        )
        return tuple(outs)

    devices = jax.devices()[:NCORES]
    mesh = Mesh(np.asarray(devices), ("core",))
    sharded = jax.jit(
        shard_map(_body, mesh=mesh,
                  in_specs=(PartitionSpec("core"),) * (n_params + n_outs),
                  out_specs=(PartitionSpec("core"),) * n_outs,
                  check_rep=False),
        donate_argnums=tuple(range(n_params, n_params + n_outs)),
        keep_unused=True,
    )

    def run(in_maps):
        concat_in = [
            np.concatenate([m[name] for m in in_maps], axis=0)
            for name in in_names
        ]
        zeros = [np.zeros((NCORES * a.shape[0], *a.shape[1:]), a.dtype)
                 for a in out_avals]
        outs = sharded(*concat_in, *zeros)
        return [
            {name: np.asarray(outs[i]).reshape(NCORES, *out_avals[i].shape)[c]
             for i, name in enumerate(out_names)}
            for c in range(NCORES)
        ]

    return run


def _in_maps(x, W, counts):
    """Host-side shard prep: xT bf16 replicated, W column-shards bf16."""
    import ml_dtypes
    bf16 = ml_dtypes.bfloat16
    xTb = x.T.astype(bf16)  # [2048, 16384], C-contig via astype copy
    maps = []
    for c in range(NCORES):
        wc = W[:, c * NSH:(c + 1) * NSH, :].transpose(0, 2, 1)
        wcb = wc.reshape(4 * HIDDEN, NSH).astype(bf16)
        maps.append({"xT": xTb, "wT": wcb})
    return maps


def kernel(x, W, modality_mapping):
    x = np.ascontiguousarray(np.asarray(x, dtype=np.float32))
    W = np.asarray(W, dtype=np.float32)
    mm = np.asarray(modality_mapping)

    perm = None
    if np.any(np.diff(mm) < 0):  # insurance: tokens not pre-sorted
        perm = np.argsort(mm, kind="stable")
        x = x[perm]
        mm = mm[perm]

    counts = tuple(int(v) for v in
                   np.bincount(mm.astype(np.int64), minlength=W.shape[0]))

    if counts not in _cache:
        _cache[counts] = _runner(counts)
    run = _cache[counts]

    results = run(_in_maps(x, W, counts))

    out = np.empty((T, QKV_OUT), dtype=np.float32)
    for c in range(NCORES):
        out[:, c * NSH:(c + 1) * NSH] = results[c]["out"].T
    if perm is not None:
        inv = np.empty_like(perm)
        inv[perm] = np.arange(T)
        out = out[inv]
    return out
